# revision 82
# baseline (speedup 1.0000x reference)
"""GuidedFilter (r=15, eps=0.5) Trainium2 Bass kernel.

Full inputs: guide, input_map [16,1,1024,1024] f32. Data-parallel over 8
NeuronCores (2 images/core). Per image:
  box(x) = Vpass(Hpass(x)) with 31-tap window sums, reflect padding.
  - H direction (free axis): tensor_tensor_scan on DVE over zero-prefixed
    padded rows (no init reduce needed; exact telescoping also lets the
    I|p and a|b tensor pairs share one wide scan each).
  - V direction (partition axis): PE band matmuls with constant bf16
    weights (reflect folded into band blocks, 1/961 normalization folded
    into the weights), fp32 PSUM accumulate.
  - eps folded into the II scan prefix (-EPS prefix -> +K*EPS per window
    sum -> +EPS on normalized mean_II out of the V-pass).
  - means pair-evacuated PSUM->SBUF bf16 on Act; r=1/(var+eps) via Act
    Abs_reciprocal_sqrt squared on DVE; coefficient chain bf16 2x on DVE.
  - stage F: box(a)/box(b) PSUM evacuated bf16 on Act, then the terminal
    out = ma*I + mb on Pool (gpsimd); Ip product and pad mirrors also on
    Pool/Act so DVE keeps only scans + chain + nothing terminal. The last
    LAG tiles instead compute o1/o2 on DVE straight from PSUM (DVE is
    idle in the epilogue; skips the evac+Pool round trip).
  - flat cross-image software pipeline (AB leads CD by 3 tiles, a|b scans
    trail CD by 1 iteration, F lags by 2) to keep all engines fed.
"""

import numpy as np
import ml_dtypes

R = 15
K = 2 * R + 1  # 31
EPS = 0.5
NORM = 1.0 / (K * K)  # 1/961

_CACHE = {}


def _build_band_weights(Hc, NT):
    """Wf[k, m] = NORM * weight of input row k in output row m's reflect window."""
    Wf = np.zeros((Hc, Hc), np.float32)
    for m in range(Hc):
        for t in range(m - R, m + R + 1):
            k = t
            if k < 0:
                k = -k
            if k > Hc - 1:
                k = 2 * (Hc - 1) - k
            Wf[k, m] += 1.0
    Wf *= NORM
    # Pack per out-tile j into [128, 3*128]:
    #   cols 0:128   = center block  (in-tile j)
    #   cols 128:256 = top edge      (in-tile j-1 rows 113:128 -> rows 64:128 window)
    #   cols 256:384 = bottom edge   (in-tile j+1 rows 0:15)
    wv = np.zeros((NT, 128, 384), np.float32)
    for j in range(NT):
        r0 = j * 128
        wv[j, :, 0:128] = Wf[r0 : r0 + 128, r0 : r0 + 128]
        if j > 0:
            wv[j, 64:128, 128:256] = Wf[r0 - 64 : r0, r0 : r0 + 128]
        if j < NT - 1:
            wv[j, 0:15, 256:384] = Wf[r0 + 128 : r0 + 143, r0 : r0 + 128]
    return wv.astype(ml_dtypes.bfloat16)


def build_nc(n_img, Hc, Wc):
    """Build the Bass module for one core processing n_img images of [Hc, Wc]."""
    import concourse.bass as bass
    import concourse.tile as tile
    from concourse import bacc, mybir

    P = 128
    NT = Hc // P
    PRE = 31              # zeros prefix so scans need no init reduce
    LM = PRE + 16         # interior start: 31 prefix + 16 left mirror
    PW = Wc + 63          # prefix + left mirror + data + right mirror + 1
    HW_ = Wc + 31         # scan output width (first 31 cols garbage)
    CH = min(512, Wc)     # psum chunk width
    NC_ = Wc // CH        # chunks per tile
    f32 = mybir.dt.float32
    bf16 = mybir.dt.bfloat16
    AX = mybir.AxisListType.X
    OP = mybir.AluOpType
    AF = mybir.ActivationFunctionType

    nc = bacc.Bacc("TRN2", target_bir_lowering=False, debug=False)
    g_dram = nc.dram_tensor("guide", [n_img, Hc, Wc], f32, kind="ExternalInput")
    p_dram = nc.dram_tensor("input_map", [n_img, Hc, Wc], f32, kind="ExternalInput")
    wv_dram = nc.dram_tensor("wv", [NT, 128, 384], bf16, kind="ExternalInput")
    o_dram = nc.dram_tensor("out", [n_img, Hc, Wc], f32, kind="ExternalOutput")
    gap, pap, wap, oap = g_dram.ap(), p_dram.ap(), wv_dram.ap(), o_dram.ap()

    with tile.TileContext(nc) as tc:
        wpool = tc.alloc_tile_pool(name="wv", bufs=1)
        wv_sb = []
        for j in range(NT):
            wt = wpool.tile([128, 384], bf16, tag=f"wv{j}", name=f"wv{j}")
            wv_sb.append(wt)
        _wv_loaded = []

        def load_wv():
            if not _wv_loaded:
                _wv_loaded.append(True)
                for j in range(NT):
                    nc.sync.dma_start(wv_sb[j][:], wap[j])

        xpi_pool = tc.alloc_tile_pool(name="xpi", bufs=8)        # guide|map combined, image-long
        xpm_pool = tc.alloc_tile_pool(name="xpm", bufs=2)        # Ip & II pads (bf16)
        h_pool = tc.alloc_tile_pool(name="hx", bufs=5)           # 4 tensors x 5
        cf_pool = tc.alloc_tile_pool(name="cf", bufs=3)          # coeff transients
        ab_pool = tc.alloc_tile_pool(name="ab", bufs=3)          # xp_a, xp_b pads (bf16)
        hab_pool = tc.alloc_tile_pool(name="hab", bufs=3)        # ha, hb
        o_pool = tc.alloc_tile_pool(name="o", bufs=2)
        mf_pool = tc.alloc_tile_pool(name="mf", bufs=2)
        ps_pool = tc.alloc_tile_pool(name="ps", bufs=1, space="PSUM")
        psab_pool = tc.alloc_tile_pool(name="psab", bufs=1, space="PSUM")

        def mirrors(xp, eng, base=0):
            # left: 16 cols before interior <- interior cols reversed; right symmetric.
            b = base
            c0 = b + LM + Wc
            if eng is nc.scalar:
                eng.copy(xp[:, b + PRE : b + PRE + 16], xp[:, b + LM + 16 : b + LM : -1])
                eng.copy(xp[:, c0 : c0 + 15], xp[:, c0 - 2 : c0 - 17 : -1])
            else:
                eng.tensor_copy(xp[:, b + PRE : b + PRE + 16], xp[:, b + LM + 16 : b + LM : -1])
                eng.tensor_copy(xp[:, c0 : c0 + 15], xp[:, c0 - 2 : c0 - 17 : -1])

        _prefix_seen = {}

        def prefix_memset(xp, tag, bufs, val, base=0, lead=0):
            """Memset the scan prefix (plus `lead` slack cols before it)
            once per ring slot of this tag."""
            n = _prefix_seen.get(tag, 0)
            if n < bufs:
                _prefix_seen[tag] = n + 1
                nc.gpsimd.memset(xp[:, base - lead : base + PRE], val)

        def hscan(xp, out, eng, width=None):
            # state(s) = sum(B[s+1 .. s+31]) - sum(first-31 prefix); prefix=0
            # (or -EPS for the II tensor: +K*EPS on every window sum). Exact
            # telescoping lets two zero-prefix tensors share one scan.
            w = (width or (PW - 1)) - PRE
            eng.tensor_tensor_scan(
                out[:, 0:w], xp[:, PRE : PRE + w], xp[:, 0:w], 0.0,
                op0=OP.add, op1=OP.subtract,
            )

        def vpass(psum, hsrc, t, c, base=0):
            """psum[128, CH] = normalized band-weighted column sums of hsrc tiles.
            t is a global tile index; the band never crosses image boundaries."""
            j = t % NT
            lo, hi = base + PRE + c * CH, base + PRE + (c + 1) * CH
            has_up = j > 0
            has_dn = j < NT - 1
            nc.tensor.matmul(
                psum[:], wv_sb[j][:, 0:128], hsrc[t][:, lo:hi],
                start=True, stop=not (has_up or has_dn),
            )
            if has_up:
                nc.tensor.matmul(
                    psum[:], wv_sb[j][64:128, 128:256], hsrc[t - 1][64:128, lo:hi],
                    start=False, stop=not has_dn,
                )
            if has_dn:
                nc.tensor.matmul(
                    psum[:], wv_sb[j][0:15, 256:384], hsrc[t + 1][0:15, lo:hi],
                    start=False, stop=True,
                )

        NTT = n_img * NT
        if True:
            xpI = [None] * NTT
            hI = [None] * NTT
            hp = [None] * NTT
            hIp = [None] * NTT
            hII = [None] * NTT
            xpa = [None] * NTT
            xpb = [None] * NTT
            ha = [None] * NTT
            hb = [None] * NTT

            def stageAB(j):
                img, jr = j // NT, j % NT
                xpI[j] = xpi_pool.tile([128, 2 * PW], f32, tag="xpI", name="xpI")
                xpIP = xpI[j]
                prefix_memset(xpIP, "xpI", 8, 0.0)
                prefix_memset(xpIP, "xpIb", 8, 0.0, base=PW, lead=1)
                nc.sync.dma_start(xpIP[:, LM : LM + Wc], gap[img, jr * 128 : (jr + 1) * 128, :])
                nc.sync.dma_start(xpIP[:, PW + LM : PW + LM + Wc], pap[img, jr * 128 : (jr + 1) * 128, :])
                mirrors(xpIP, nc.scalar)
                mirrors(xpIP, nc.scalar, base=PW)
                xpIp = xpm_pool.tile([128, PW], bf16, tag="xpIp", name="xpIp")
                xpII = xpm_pool.tile([128, PW], bf16, tag="xpII", name="xpII")
                prefix_memset(xpIp, "xpIp", 2, 0.0)
                # -EPS prefix adds +K*EPS to every hII window sum -> +EPS on mean_II
                prefix_memset(xpII, "xpII", 2, -EPS)
                nc.gpsimd.tensor_tensor(
                    xpIp[:, LM : LM + Wc], xpIP[:, LM : LM + Wc],
                    xpIP[:, PW + LM : PW + LM + Wc], op=OP.mult,
                )
                nc.scalar.activation(
                    xpII[:, LM : LM + Wc], xpIP[:, LM : LM + Wc], AF.Square
                )
                mirrors(xpIp, nc.gpsimd)
                mirrors(xpII, nc.gpsimd)
                hI[j] = h_pool.tile([128, 2 * PW], bf16, tag="hIP", name="hIP")
                hp[j] = hI[j]
                hIp[j] = h_pool.tile([128, HW_], bf16, tag="hIp", name="hIp")
                hII[j] = h_pool.tile([128, HW_], bf16, tag="hII", name="hII")
                hscan(xpIP, hI[j], nc.vector, width=2 * PW)
                hscan(xpIp, hIp[j], nc.vector)
                hscan(xpII, hII[j], nc.vector)

            def stageCD(j):
                xpa[j] = ab_pool.tile([128, 2 * PW], bf16, tag="xpab", name="xpab")
                xpb[j] = xpa[j]
                prefix_memset(xpa[j], "xpa", 3, 0.0)
                prefix_memset(xpa[j], "xpb", 3, 0.0, base=PW, lead=1)
                st = []
                for c in range(NC_):
                    psAB = ps_pool.tile([128, 2 * CH], f32, tag="psAB", name="psAB")
                    psCD = ps_pool.tile([128, 2 * CH], f32, tag="psCD", name="psCD")
                    vpass(psAB[:, 0:CH], hI, j, c)
                    vpass(psAB[:, CH : 2 * CH], hp, j, c, base=PW)
                    vpass(psCD[:, 0:CH], hIp, j, c)
                    vpass(psCD[:, CH : 2 * CH], hII, j, c)
                    # paired evacuation of the 4 means (frees PSUM fast,
                    # enables bf16 2x DVE ops downstream)
                    mab = cf_pool.tile([128, 2 * CH], bf16, tag="mab", name="mab")
                    nc.scalar.copy(mab[:], psAB[:])
                    mcd = cf_pool.tile([128, 2 * CH], bf16, tag="mcd", name="mcd")
                    nc.scalar.copy(mcd[:], psCD[:])
                    st.append((mab, mcd))
                sqs, covs, rps = [], [], []
                for c in range(NC_):
                    mab, mcd = st[c]
                    sq = cf_pool.tile([128, CH], bf16, tag="sq", name="sq")
                    nc.scalar.activation(sq[:], mab[:, 0:CH], AF.Square)
                    sqs.append(sq)
                    prod = cf_pool.tile([128, CH], bf16, tag="prod", name="prod")
                    nc.vector.tensor_tensor(prod[:], mab[:, 0:CH], mab[:, CH:], op=OP.mult)
                    cov = cf_pool.tile([128, CH], bf16, tag="cov", name="cov")
                    nc.vector.tensor_tensor(cov[:], mcd[:, 0:CH], prod[:], op=OP.subtract)
                    covs.append(cov)
                for c in range(NC_):
                    mab, mcd = st[c]
                    den = cf_pool.tile([128, CH], bf16, tag="den", name="den")
                    nc.vector.tensor_tensor(den[:], mcd[:, CH:], sqs[c][:], op=OP.subtract)
                    rp = cf_pool.tile([128, CH], bf16, tag="rp", name="rp")
                    nc.scalar.activation(rp[:], den[:], AF.Abs_reciprocal_sqrt)
                    rps.append(rp)
                for c in range(NC_):
                    mab, mcd = st[c]
                    rp = rps[c]
                    u = cf_pool.tile([128, CH], bf16, tag="u", name="u")
                    nc.vector.tensor_tensor(u[:], covs[c][:], rp[:], op=OP.mult)
                    lo = LM + c * CH
                    av = xpa[j][:, lo : lo + CH]
                    nc.vector.tensor_tensor(av, u[:], rp[:], op=OP.mult)
                    t = cf_pool.tile([128, CH], bf16, tag="t", name="t")
                    nc.vector.tensor_tensor(t[:], av, mab[:, 0:CH], op=OP.mult)
                    nc.vector.tensor_tensor(
                        xpb[j][:, PW + lo : PW + lo + CH], mab[:, CH:], t[:], op=OP.subtract
                    )
                mirrors(xpa[j], nc.scalar)
                mirrors(xpa[j], nc.scalar, base=PW)

            def stageCDscan(j):
                ha[j] = hab_pool.tile([128, 2 * PW], bf16, tag="hab", name="hab")
                hb[j] = ha[j]
                hscan(xpa[j], ha[j], nc.vector, width=2 * PW)

            def stageF(j, on_dve=False):
                img, jr = j // NT, j % NT
                psa = psab_pool.tile([128, Wc], f32, tag="psa", name="psa")
                for c in range(NC_):
                    vpass(psa[:, c * CH : (c + 1) * CH], ha, j, c)
                if not on_dve:
                    mFa = mf_pool.tile([128, Wc], bf16, tag="mFa", name="mFa")
                    nc.scalar.copy(mFa[:], psa[:])
                psb = psab_pool.tile([128, Wc], f32, tag="psb", name="psb")
                for c in range(NC_):
                    vpass(psb[:, c * CH : (c + 1) * CH], hb, j, c, base=PW)
                o1 = o_pool.tile([128, Wc], f32, tag="o1", name="o1")
                o2 = o_pool.tile([128, Wc], f32, tag="o2", name="o2")
                if on_dve:
                    # epilogue: DVE is idle here; skip the evac+Pool round trip
                    nc.vector.tensor_tensor(
                        o1[:], psa[:], xpI[j][:, LM : LM + Wc], op=OP.mult
                    )
                    nc.vector.tensor_tensor(o2[:], o1[:], psb[:], op=OP.add)
                else:
                    mFb = mf_pool.tile([128, Wc], bf16, tag="mFb", name="mFb")
                    nc.scalar.copy(mFb[:], psb[:])
                    nc.gpsimd.tensor_tensor(
                        o1[:], mFa[:], xpI[j][:, LM : LM + Wc], op=OP.mult
                    )
                    nc.gpsimd.tensor_tensor(o2[:], o1[:], mFb[:], op=OP.add)
                nc.sync.dma_start(
                    oap[img, jr * 128 : (jr + 1) * 128, :], o2[:]
                )

        # flat software pipeline across all images: AB leads CD by LEAD tiles,
        # a|b scans trail CD by 1, F lags CD by LAG.
        LEAD = 2
        LAG = 2
        stageAB(0)
        load_wv()
        for jj in range(1, min(LEAD, NTT)):
            stageAB(jj)
        for t in range(NTT):
            stageCD(t)
            if t >= 1:
                stageCDscan(t - 1)
            if t >= LAG:
                stageF(t - LAG)
            if t + LEAD < NTT:
                stageAB(t + LEAD)
        stageCDscan(NTT - 1)
        for t in range(NTT - LAG, NTT):
            stageF(t, on_dve=True)

        for _pool in (psab_pool, ps_pool, mf_pool, o_pool, hab_pool, ab_pool,
                      cf_pool, h_pool, xpm_pool, xpi_pool, wpool):
            _pool.release()

    nc.compile()
    return nc


def _get_nc(n_img, Hc, Wc):
    key = (n_img, Hc, Wc)
    if key not in _CACHE:
        _CACHE[key] = build_nc(n_img, Hc, Wc)
    return _CACHE[key]


def kernel(guide, input_map):
    from concourse.bass_utils import run_bass_kernel_spmd

    B, C, Hc, Wc = guide.shape
    n_cores = 8
    n_img = B // n_cores
    g = np.ascontiguousarray(guide.reshape(B, Hc, Wc), dtype=np.float32)
    p = np.ascontiguousarray(input_map.reshape(B, Hc, Wc), dtype=np.float32)
    wv = _build_band_weights(Hc, Hc // 128)
    nc = _get_nc(n_img, Hc, Wc)
    in_maps = [
        {
            "guide": g[i * n_img : (i + 1) * n_img],
            "input_map": p[i * n_img : (i + 1) * n_img],
            "wv": wv,
        }
        for i in range(n_cores)
    ]
    res = run_bass_kernel_spmd(nc, in_maps, core_ids=list(range(n_cores)))
    out = np.concatenate([res.results[i]["out"] for i in range(n_cores)], axis=0)
    return out.reshape(B, C, Hc, Wc).astype(np.float32)


# revision 83
# speedup vs baseline: 1.0034x; 1.0034x over previous
"""GuidedFilter (r=15, eps=0.5) Trainium2 Bass kernel.

Full inputs: guide, input_map [16,1,1024,1024] f32. Data-parallel over 8
NeuronCores (2 images/core). Per image:
  box(x) = Vpass(Hpass(x)) with 31-tap window sums, reflect padding.
  - H direction (free axis): tensor_tensor_scan on DVE over zero-prefixed
    padded rows (no init reduce needed; exact telescoping also lets the
    I|p and a|b tensor pairs share one wide scan each).
  - V direction (partition axis): PE band matmuls with constant bf16
    weights (reflect folded into band blocks, 1/961 normalization folded
    into the weights), fp32 PSUM accumulate.
  - eps folded into the II scan prefix (-EPS prefix -> +K*EPS per window
    sum -> +EPS on normalized mean_II out of the V-pass).
  - means pair-evacuated PSUM->SBUF bf16 on Act; r=1/(var+eps) via Act
    Abs_reciprocal_sqrt squared on DVE; coefficient chain bf16 2x on DVE.
  - stage F: box(a)/box(b) PSUM evacuated bf16 on Act, then the terminal
    out = ma*I + mb on Pool (gpsimd); Ip product and pad mirrors also on
    Pool/Act so DVE keeps only scans + chain + nothing terminal. The last
    LAG tiles instead compute o1/o2 on DVE straight from PSUM (DVE is
    idle in the epilogue; skips the evac+Pool round trip).
  - flat cross-image software pipeline (AB leads CD by 3 tiles, a|b scans
    trail CD by 1 iteration, F lags by 2) to keep all engines fed.
"""

import numpy as np
import ml_dtypes

R = 15
K = 2 * R + 1  # 31
EPS = 0.5
NORM = 1.0 / (K * K)  # 1/961

_CACHE = {}


def _build_band_weights(Hc, NT):
    """Wf[k, m] = NORM * weight of input row k in output row m's reflect window."""
    Wf = np.zeros((Hc, Hc), np.float32)
    for m in range(Hc):
        for t in range(m - R, m + R + 1):
            k = t
            if k < 0:
                k = -k
            if k > Hc - 1:
                k = 2 * (Hc - 1) - k
            Wf[k, m] += 1.0
    Wf *= NORM
    # Pack per out-tile j into [128, 3*128]:
    #   cols 0:128   = center block  (in-tile j)
    #   cols 128:256 = top edge      (in-tile j-1 rows 113:128 -> rows 64:128 window)
    #   cols 256:384 = bottom edge   (in-tile j+1 rows 0:15)
    wv = np.zeros((NT, 128, 384), np.float32)
    for j in range(NT):
        r0 = j * 128
        wv[j, :, 0:128] = Wf[r0 : r0 + 128, r0 : r0 + 128]
        if j > 0:
            wv[j, 64:128, 128:256] = Wf[r0 - 64 : r0, r0 : r0 + 128]
        if j < NT - 1:
            wv[j, 0:15, 256:384] = Wf[r0 + 128 : r0 + 143, r0 : r0 + 128]
    return wv.astype(ml_dtypes.bfloat16)


def build_nc(n_img, Hc, Wc):
    """Build the Bass module for one core processing n_img images of [Hc, Wc]."""
    import concourse.bass as bass
    import concourse.tile as tile
    from concourse import bacc, mybir

    P = 128
    NT = Hc // P
    PRE = 31              # zeros prefix so scans need no init reduce
    LM = PRE + 16         # interior start: 31 prefix + 16 left mirror
    PW = Wc + 63          # prefix + left mirror + data + right mirror + 1
    HW_ = Wc + 31         # scan output width (first 31 cols garbage)
    CH = min(512, Wc)     # psum chunk width
    NC_ = Wc // CH        # chunks per tile
    f32 = mybir.dt.float32
    bf16 = mybir.dt.bfloat16
    AX = mybir.AxisListType.X
    OP = mybir.AluOpType
    AF = mybir.ActivationFunctionType

    nc = bacc.Bacc("TRN2", target_bir_lowering=False, debug=False)
    g_dram = nc.dram_tensor("guide", [n_img, Hc, Wc], f32, kind="ExternalInput")
    p_dram = nc.dram_tensor("input_map", [n_img, Hc, Wc], f32, kind="ExternalInput")
    wv_dram = nc.dram_tensor("wv", [NT, 128, 384], bf16, kind="ExternalInput")
    o_dram = nc.dram_tensor("out", [n_img, Hc, Wc], f32, kind="ExternalOutput")
    gap, pap, wap, oap = g_dram.ap(), p_dram.ap(), wv_dram.ap(), o_dram.ap()

    with tile.TileContext(nc) as tc:
        wpool = tc.alloc_tile_pool(name="wv", bufs=1)
        wv_sb = []
        for j in range(NT):
            wt = wpool.tile([128, 384], bf16, tag=f"wv{j}", name=f"wv{j}")
            wv_sb.append(wt)
        _wv_loaded = []

        def load_wv():
            if not _wv_loaded:
                _wv_loaded.append(True)
                for j in range(NT):
                    nc.sync.dma_start(wv_sb[j][:], wap[j])

        xpi_pool = tc.alloc_tile_pool(name="xpi", bufs=8)        # guide|map combined, image-long
        xpm_pool = tc.alloc_tile_pool(name="xpm", bufs=2)        # Ip & II pads (bf16)
        h_pool = tc.alloc_tile_pool(name="hx", bufs=5)           # 4 tensors x 5
        cf_pool = tc.alloc_tile_pool(name="cf", bufs=3)          # coeff transients
        ab_pool = tc.alloc_tile_pool(name="ab", bufs=3)          # xp_a, xp_b pads (bf16)
        hab_pool = tc.alloc_tile_pool(name="hab", bufs=3)        # ha, hb
        o_pool = tc.alloc_tile_pool(name="o", bufs=2)
        mf_pool = tc.alloc_tile_pool(name="mf", bufs=2)
        ps_pool = tc.alloc_tile_pool(name="ps", bufs=1, space="PSUM")
        psab_pool = tc.alloc_tile_pool(name="psab", bufs=1, space="PSUM")

        def mirrors(xp, eng, base=0):
            # left: 16 cols before interior <- interior cols reversed; right symmetric.
            b = base
            c0 = b + LM + Wc
            if eng is nc.scalar:
                eng.copy(xp[:, b + PRE : b + PRE + 16], xp[:, b + LM + 16 : b + LM : -1])
                eng.copy(xp[:, c0 : c0 + 15], xp[:, c0 - 2 : c0 - 17 : -1])
            else:
                eng.tensor_copy(xp[:, b + PRE : b + PRE + 16], xp[:, b + LM + 16 : b + LM : -1])
                eng.tensor_copy(xp[:, c0 : c0 + 15], xp[:, c0 - 2 : c0 - 17 : -1])

        _prefix_seen = {}

        def prefix_memset(xp, tag, bufs, val, base=0, lead=0):
            """Memset the scan prefix (plus `lead` slack cols before it)
            once per ring slot of this tag."""
            n = _prefix_seen.get(tag, 0)
            if n < bufs:
                _prefix_seen[tag] = n + 1
                nc.gpsimd.memset(xp[:, base - lead : base + PRE], val)

        def hscan(xp, out, eng, width=None):
            # state(s) = sum(B[s+1 .. s+31]) - sum(first-31 prefix); prefix=0
            # (or -EPS for the II tensor: +K*EPS on every window sum). Exact
            # telescoping lets two zero-prefix tensors share one scan.
            w = (width or (PW - 1)) - PRE
            eng.tensor_tensor_scan(
                out[:, 0:w], xp[:, PRE : PRE + w], xp[:, 0:w], 0.0,
                op0=OP.add, op1=OP.subtract,
            )

        def vpass(psum, hsrc, t, c, base=0):
            """psum[128, CH] = normalized band-weighted column sums of hsrc tiles.
            t is a global tile index; the band never crosses image boundaries."""
            j = t % NT
            lo, hi = base + PRE + c * CH, base + PRE + (c + 1) * CH
            has_up = j > 0
            has_dn = j < NT - 1
            nc.tensor.matmul(
                psum[:], wv_sb[j][:, 0:128], hsrc[t][:, lo:hi],
                start=True, stop=not (has_up or has_dn),
            )
            if has_up:
                nc.tensor.matmul(
                    psum[:], wv_sb[j][64:128, 128:256], hsrc[t - 1][64:128, lo:hi],
                    start=False, stop=not has_dn,
                )
            if has_dn:
                nc.tensor.matmul(
                    psum[:], wv_sb[j][0:15, 256:384], hsrc[t + 1][0:15, lo:hi],
                    start=False, stop=True,
                )

        NTT = n_img * NT
        if True:
            xpI = [None] * NTT
            hI = [None] * NTT
            hp = [None] * NTT
            hIp = [None] * NTT
            hII = [None] * NTT
            xpa = [None] * NTT
            xpb = [None] * NTT
            ha = [None] * NTT
            hb = [None] * NTT

            def stageAB(j):
                img, jr = j // NT, j % NT
                xpI[j] = xpi_pool.tile([128, 2 * PW], f32, tag="xpI", name="xpI")
                xpIP = xpI[j]
                prefix_memset(xpIP, "xpI", 8, 0.0)
                prefix_memset(xpIP, "xpIb", 8, 0.0, base=PW, lead=1)
                nc.sync.dma_start(xpIP[:, LM : LM + Wc], gap[img, jr * 128 : (jr + 1) * 128, :])
                nc.sync.dma_start(xpIP[:, PW + LM : PW + LM + Wc], pap[img, jr * 128 : (jr + 1) * 128, :])
                mirrors(xpIP, nc.scalar)
                mirrors(xpIP, nc.scalar, base=PW)
                xpIp = xpm_pool.tile([128, PW], bf16, tag="xpIp", name="xpIp")
                xpII = xpm_pool.tile([128, PW], bf16, tag="xpII", name="xpII")
                prefix_memset(xpIp, "xpIp", 2, 0.0)
                # -EPS prefix adds +K*EPS to every hII window sum -> +EPS on mean_II
                prefix_memset(xpII, "xpII", 2, -EPS)
                nc.gpsimd.tensor_tensor(
                    xpIp[:, LM : LM + Wc], xpIP[:, LM : LM + Wc],
                    xpIP[:, PW + LM : PW + LM + Wc], op=OP.mult,
                )
                nc.scalar.activation(
                    xpII[:, LM : LM + Wc], xpIP[:, LM : LM + Wc], AF.Square
                )
                mirrors(xpIp, nc.gpsimd)
                mirrors(xpII, nc.gpsimd)
                hI[j] = h_pool.tile([128, 2 * PW], bf16, tag="hIP", name="hIP")
                hp[j] = hI[j]
                hIp[j] = h_pool.tile([128, HW_], bf16, tag="hIp", name="hIp")
                hII[j] = h_pool.tile([128, HW_], bf16, tag="hII", name="hII")
                hscan(xpIP, hI[j], nc.vector, width=2 * PW)
                hscan(xpIp, hIp[j], nc.vector)
                hscan(xpII, hII[j], nc.vector)

            def stageCD(j):
                xpa[j] = ab_pool.tile([128, 2 * PW], bf16, tag="xpab", name="xpab")
                xpb[j] = xpa[j]
                prefix_memset(xpa[j], "xpa", 3, 0.0)
                prefix_memset(xpa[j], "xpb", 3, 0.0, base=PW, lead=1)
                st = []
                for c in range(NC_):
                    psAB = ps_pool.tile([128, 2 * CH], f32, tag="psAB", name="psAB")
                    psCD = ps_pool.tile([128, 2 * CH], f32, tag="psCD", name="psCD")
                    vpass(psAB[:, 0:CH], hI, j, c)
                    vpass(psAB[:, CH : 2 * CH], hp, j, c, base=PW)
                    vpass(psCD[:, 0:CH], hIp, j, c)
                    vpass(psCD[:, CH : 2 * CH], hII, j, c)
                    # paired evacuation of the 4 means (frees PSUM fast,
                    # enables bf16 2x DVE ops downstream)
                    mab = cf_pool.tile([128, 2 * CH], bf16, tag="mab", name="mab")
                    nc.scalar.copy(mab[:], psAB[:])
                    mcd = cf_pool.tile([128, 2 * CH], bf16, tag="mcd", name="mcd")
                    nc.scalar.copy(mcd[:], psCD[:])
                    st.append((mab, mcd))
                sqs, covs, rps = [], [], []
                for c in range(NC_):
                    mab, mcd = st[c]
                    sq = cf_pool.tile([128, CH], bf16, tag="sq", name="sq")
                    nc.scalar.activation(sq[:], mab[:, 0:CH], AF.Square)
                    sqs.append(sq)
                    prod = cf_pool.tile([128, CH], bf16, tag="prod", name="prod")
                    nc.vector.tensor_tensor(prod[:], mab[:, 0:CH], mab[:, CH:], op=OP.mult)
                    cov = cf_pool.tile([128, CH], bf16, tag="cov", name="cov")
                    nc.vector.tensor_tensor(cov[:], mcd[:, 0:CH], prod[:], op=OP.subtract)
                    covs.append(cov)
                for c in range(NC_):
                    mab, mcd = st[c]
                    den = cf_pool.tile([128, CH], bf16, tag="den", name="den")
                    nc.vector.tensor_tensor(den[:], mcd[:, CH:], sqs[c][:], op=OP.subtract)
                    rp = cf_pool.tile([128, CH], bf16, tag="rp", name="rp")
                    nc.scalar.activation(rp[:], den[:], AF.Abs_reciprocal_sqrt)
                    rps.append(rp)
                for c in range(NC_):
                    mab, mcd = st[c]
                    rp = rps[c]
                    u = cf_pool.tile([128, CH], bf16, tag="u", name="u")
                    nc.vector.tensor_tensor(u[:], covs[c][:], rp[:], op=OP.mult)
                    lo = LM + c * CH
                    av = xpa[j][:, lo : lo + CH]
                    nc.vector.tensor_tensor(av, u[:], rp[:], op=OP.mult)
                    t = cf_pool.tile([128, CH], bf16, tag="t", name="t")
                    nc.vector.tensor_tensor(t[:], av, mab[:, 0:CH], op=OP.mult)
                    nc.vector.tensor_tensor(
                        xpb[j][:, PW + lo : PW + lo + CH], mab[:, CH:], t[:], op=OP.subtract
                    )
                mirrors(xpa[j], nc.scalar)
                mirrors(xpa[j], nc.scalar, base=PW)

            def stageCDscan(j):
                ha[j] = hab_pool.tile([128, 2 * PW], bf16, tag="hab", name="hab")
                hb[j] = ha[j]
                hscan(xpa[j], ha[j], nc.vector, width=2 * PW)

            def stageF(j, on_dve=False):
                img, jr = j // NT, j % NT
                psa = psab_pool.tile([128, Wc], f32, tag="psa", name="psa")
                for c in range(NC_):
                    vpass(psa[:, c * CH : (c + 1) * CH], ha, j, c)
                if not on_dve:
                    mFa = mf_pool.tile([128, Wc], bf16, tag="mFa", name="mFa")
                    nc.scalar.copy(mFa[:], psa[:])
                psb = psab_pool.tile([128, Wc], f32, tag="psb", name="psb")
                for c in range(NC_):
                    vpass(psb[:, c * CH : (c + 1) * CH], hb, j, c, base=PW)
                o1 = o_pool.tile([128, Wc], f32, tag="o1", name="o1")
                o2 = o_pool.tile([128, Wc], f32, tag="o2", name="o2")
                if on_dve:
                    # epilogue: DVE is idle here; skip the evac+Pool round trip
                    nc.vector.tensor_tensor(
                        o1[:], psa[:], xpI[j][:, LM : LM + Wc], op=OP.mult
                    )
                    nc.vector.tensor_tensor(o2[:], o1[:], psb[:], op=OP.add)
                else:
                    mFb = mf_pool.tile([128, Wc], bf16, tag="mFb", name="mFb")
                    nc.scalar.copy(mFb[:], psb[:])
                    nc.gpsimd.tensor_tensor(
                        o1[:], mFa[:], xpI[j][:, LM : LM + Wc], op=OP.mult
                    )
                    nc.gpsimd.tensor_tensor(o2[:], o1[:], mFb[:], op=OP.add)
                nc.sync.dma_start(
                    oap[img, jr * 128 : (jr + 1) * 128, :], o2[:]
                )

        # flat software pipeline across all images: AB leads CD by LEAD tiles,
        # a|b scans trail CD by 1, F lags CD by LAG.
        LEAD = 3
        LAG = 2
        stageAB(0)
        load_wv()
        for jj in range(1, min(LEAD, NTT)):
            stageAB(jj)
        for t in range(NTT):
            stageCD(t)
            if t >= 1:
                stageCDscan(t - 1)
            if t >= LAG:
                stageF(t - LAG)
            if t + LEAD < NTT:
                stageAB(t + LEAD)
        stageCDscan(NTT - 1)
        for t in range(NTT - LAG, NTT):
            stageF(t, on_dve=True)

        for _pool in (psab_pool, ps_pool, mf_pool, o_pool, hab_pool, ab_pool,
                      cf_pool, h_pool, xpm_pool, xpi_pool, wpool):
            _pool.release()

    nc.compile()
    return nc


def _get_nc(n_img, Hc, Wc):
    key = (n_img, Hc, Wc)
    if key not in _CACHE:
        _CACHE[key] = build_nc(n_img, Hc, Wc)
    return _CACHE[key]


def kernel(guide, input_map):
    from concourse.bass_utils import run_bass_kernel_spmd

    B, C, Hc, Wc = guide.shape
    n_cores = 8
    n_img = B // n_cores
    g = np.ascontiguousarray(guide.reshape(B, Hc, Wc), dtype=np.float32)
    p = np.ascontiguousarray(input_map.reshape(B, Hc, Wc), dtype=np.float32)
    wv = _build_band_weights(Hc, Hc // 128)
    nc = _get_nc(n_img, Hc, Wc)
    in_maps = [
        {
            "guide": g[i * n_img : (i + 1) * n_img],
            "input_map": p[i * n_img : (i + 1) * n_img],
            "wv": wv,
        }
        for i in range(n_cores)
    ]
    res = run_bass_kernel_spmd(nc, in_maps, core_ids=list(range(n_cores)))
    out = np.concatenate([res.results[i]["out"] for i in range(n_cores)], axis=0)
    return out.reshape(B, C, Hc, Wc).astype(np.float32)
